# revision 25
# baseline (speedup 1.0000x reference)
"""MoE (top-2 of 8 experts) Trainium2 kernel — fp8 DoubleRow edition.

Strategy: expert-parallel across the 8 NeuronCores (host router builds the
dispatch; core e gets the tokens routed to expert e). The expert MLP runs
as 3-term residual-split fp8 matmuls in DoubleRow perf mode (contraction
256 per pass, 0.5 PE cycles per output column — 4x the bf16 MAC rate):

  A ≈ Ah + Al  (Ah = e4m3(s*A), Al = e4m3(s*A - Ah): the RN residual is
  exactly representable in e4m3, so Ah+Al carries ~9 significant bits)

  A@B ≈ Ah@Bh + Ah@Bl + Al@Bh   (lo*lo term dropped, ~1e-3 rel)

Per-core pipeline (count tokens, chunks of <=512 on the free axis):
  stage 1: ps1 = 16*z = sum of 3 DoubleRow terms (w1 planes x x planes)
           sg = sigmoid(ps1/16)         [ACT]
           t  = ps1*sg = 16*silu(z)     [DVE, bf16]
           Hh = e4m3(t)                 [ACT copy]
           Hl = e4m3(t - Hh)            [DVE fused (t*1)-Hh]
  stage 2: ps2 = 512*y = 3 DoubleRow terms (w2 planes x H planes),
           I padded to 1536 (12 i-tiles) with zero weights/h
           yv = bf16(ps2)               [ACT copy] -> DMA out
  host:    out[tok] += gate/512 * yv.T  (gate multiply + dequant on host)

All DRAM operands are host-tiled so every DMA lands with >=512B
contiguous runs and block granularity matching PE consumption order.
"""

import numpy as np
import ml_dtypes

import concourse.mybir as mybir
from concourse import bacc
from concourse.tile import TileContext
from concourse.bass_utils import run_bass_kernel_spmd

T, H, I, E = 4096, 1024, 1408, 8
TOPK = 2
P = 128
HK = H // P  # 8 h-tiles (stage-1 contraction, stage-2 output)
IT = I // P  # 11 i-tiles (stage-1 output)
IT2 = IT + 1  # stage-2 contraction padded to 12 tiles (6 DoubleRow pairs)
CHUNK = 512
N_CORES = 8

F32 = mybir.dt.float32
BF16 = mybir.dt.bfloat16
F8 = mybir.dt.float8e4
AF = mybir.ActivationFunctionType
DR = mybir.MatmulPerfMode.DoubleRow
E4NP = ml_dtypes.float8_e4m3

WARMUP_MM = 16  # PE warm-up matmuls (see build_moe_expert_kernel)
SX = 2.0  # x plane scale
SW1 = 8.0  # w1 plane scale ->  ps1 = 16*z
SW2 = 32.0  # w2 plane scale ->  ps2 = 512*y
PS1_SCALE = SX * SW1
PS2_SCALE = PS1_SCALE * SW2

# most recently built device program (for test harnesses / cost-model timing)
LAST_NC = None


def _chunks(count):
    out = []
    r = count
    while r > 0:
        c = min(CHUNK, r)
        out.append(c)
        r -= c
    # keep a full-size chunk LAST: its stage-2 compute covers the per-ht
    # output-DMA issue slots, so only one small DMA sits in the tail
    if len(out) >= 2 and out[-1] < CHUNK:
        out[-1], out[-2] = out[-2], out[-1]
    return out


def build_moe_expert_kernel(count):
    """One-expert MLP over `count` gathered tokens (count even)."""
    C = count
    assert count % 2 == 0
    c_chunks = _chunks(count)
    c_starts = [sum(c_chunks[:j]) for j in range(len(c_chunks))]
    n_chunks = len(c_chunks)

    nc = bacc.Bacc("TRN2", target_bir_lowering=False, debug=False, num_devices=N_CORES)
    # hi/lo fp8 planes packed in one tensor each: a single DMA delivers
    # both (HWDGE issue slots, 625ns apiece, are the scarce resource)
    x = nc.dram_tensor("x", [P, 2, HK, C], F8, kind="ExternalInput").ap()
    w1 = nc.dram_tensor("w1", [IT, P, 2, HK, P], F8, kind="ExternalInput").ap()
    w2 = nc.dram_tensor("w2", [HK, P, 2, IT2, P], F8, kind="ExternalInput").ap()
    y = nc.dram_tensor("y", [P, HK, C], BF16, kind="ExternalOutput").ap()

    with TileContext(nc) as tc:
        with (
            tc.tile_pool(name="wpool", bufs=1) as wpool,
            tc.tile_pool(name="xpool", bufs=min(3, n_chunks)) as xpool,
            tc.tile_pool(name="hpool", bufs=min(2, n_chunks)) as hpool,
            tc.tile_pool(name="spool", bufs=4) as spool,
            tc.tile_pool(name="ypool", bufs=min(2, n_chunks)) as ypool,
            tc.tile_pool(name="ps1", bufs=4, space="PSUM") as ps1pool,
            tc.tile_pool(name="ps2", bufs=4, space="PSUM") as ps2pool,
        ):
            w1s = wpool.tile([P, IT, 2, HK, P], F8)
            w2s = wpool.tile([P, HK, 2, IT2, P], F8)
            # scratch operand for PE warm-up matmuls (zeroed once by DVE)
            wu = wpool.tile([P, 2, CHUNK], F8)

            x_t = {}

            def load_x(ci, slabs=False):
                # per-plane DMAs: a plane slice keeps the access pattern
                # within the 3-dim DMA balancing limit
                cs, c0 = c_chunks[ci], c_starts[ci]
                tx = xpool.tile([P, 2, HK, CHUNK], F8, tag="xs", name=f"xs{ci}")
                if slabs:
                    # hk-halves per plane, interleaved, so the first
                    # stage-1 group's terms unblock progressively
                    for k in range(2):
                        sl = slice(4 * k, 4 * k + 4)
                        nc.sync.dma_start(
                            tx[:, 0, sl, :cs], x[:, 0, sl, c0 : c0 + cs]
                        )
                        nc.sync.dma_start(
                            tx[:, 1, sl, :cs], x[:, 1, sl, c0 : c0 + cs]
                        )
                else:
                    for pl in range(2):
                        nc.sync.dma_start(
                            tx[:, pl, :, :cs], x[:, pl, :, c0 : c0 + cs]
                        )
                x_t[ci] = tx

            # DMA issue order = consumption order: w1 it0, the first x
            # chunk in slabs, the w1 stream (later x chunks slotted in
            # where the stall-free window allows), then w2.
            nc.sync.dma_start(w1s[:, 0, :, :, :], w1[0])
            load_x(0, slabs=True)
            for it in range(1, IT):
                nc.sync.dma_start(w1s[:, it, :, :, :], w1[it])
                if it == 4:
                    for ci in range(1, n_chunks):
                        load_x(ci)
            for ht in range(HK):
                nc.sync.dma_start(w2s[:, ht, :, :, :], w2[ht])

            # PE p-state warm-up: the cost model runs the PE at reduced
            # clock for the first ~3us of a busy period. Dummy matmuls on
            # scratch data (discarded psum) keep the PE continuously busy
            # from t~0 so the ramp completes before real operands land.
            nc.vector.memset(wu[:, :, :], 0.0)
            wu_ps = ps1pool.tile([P, CHUNK], F32, tag="ps1")
            for _ in range(WARMUP_MM):
                nc.tensor.matmul(
                    wu_ps[:, :CHUNK],
                    wu[:, :, :P],
                    wu[:, :, :],
                    start=True,
                    stop=True,
                    perf_mode=DR,
                )

            hh_t, hl_t = {}, {}

            def stage1(ci):
                cs = c_chunks[ci]
                xs = x_t[ci]
                hh = hpool.tile([P, IT2, CHUNK], F8, tag="hh", name=f"hh{ci}")
                hl = hpool.tile([P, IT2, CHUNK], F8, tag="hl", name=f"hl{ci}")
                # stage-2 reads the zero-pad i-tile via its DoubleRow pair.
                # Only the first rotation of each pool buffer needs the
                # memset: the pad slice is never written afterwards, and we
                # zero the full CHUNK width so shorter tail chunks reusing
                # the buffer inherit zeros.
                if ci < min(2, n_chunks):
                    nc.gpsimd.memset(hh[:, IT, :], 0.0)
                    nc.gpsimd.memset(hl[:, IT, :], 0.0)
                for it in range(IT):
                    ps1 = ps1pool.tile([P, CHUNK], F32, tag="ps1")
                    terms = ((0, 0), (0, 1), (1, 0))  # (w-plane, x-plane)
                    for ti, (wp, xp) in enumerate(terms):
                        for k in range(HK // 2):
                            nc.tensor.matmul(
                                ps1[:, :cs],
                                w1s[:, it, wp, 2 * k : 2 * k + 2, :],
                                xs[:, xp, 2 * k : 2 * k + 2, :cs],
                                start=(ti == 0 and k == 0),
                                stop=(ti == 2 and k == HK // 2 - 1),
                                perf_mode=DR,
                            )
                    sg = spool.tile([P, CHUNK], F32, tag="sg")
                    t = spool.tile([P, CHUNK], BF16, tag="t")
                    nc.scalar.activation(
                        sg[:, :cs], ps1[:, :cs], AF.Sigmoid, scale=1.0 / PS1_SCALE
                    )
                    nc.vector.tensor_mul(out=t[:, :cs], in0=ps1[:, :cs], in1=sg[:, :cs])
                    nc.scalar.activation(hh[:, it, :cs], t[:, :cs], AF.Copy)
                    # residual on GPSIMD: keeps the DVE (which paces ps1
                    # reuse) well under the PE rate during stage 1
                    nc.gpsimd.scalar_tensor_tensor(
                        out=hl[:, it, :cs],
                        in0=t[:, :cs],
                        scalar=1.0,
                        in1=hh[:, it, :cs],
                        op0=mybir.AluOpType.mult,
                        op1=mybir.AluOpType.subtract,
                    )
                hh_t[ci], hl_t[ci] = hh, hl

            def stage2(ci):
                cs, c0 = c_chunks[ci], c_starts[ci]
                hh, hl = hh_t.pop(ci), hl_t.pop(ci)
                yv = ypool.tile([P, HK, CHUNK], BF16, tag="yv", name=f"yv{ci}")
                for ht in range(HK):
                    ps2 = ps2pool.tile([P, CHUNK], F32, tag="ps2")
                    terms = ((0, hh), (0, hl), (1, hh))  # (w-plane, h-plane)
                    for ti, (wp, hs) in enumerate(terms):
                        for k in range(IT2 // 2):
                            nc.tensor.matmul(
                                ps2[:, :cs],
                                w2s[:, ht, wp, 2 * k : 2 * k + 2, :],
                                hs[:, 2 * k : 2 * k + 2, :cs],
                                start=(ti == 0 and k == 0),
                                stop=(ti == 2 and k == IT2 // 2 - 1),
                                perf_mode=DR,
                            )
                    nc.scalar.activation(yv[:, ht, :cs], ps2[:, :cs], AF.Copy)
                    # per-ht output DMAs drain under the remaining groups
                    nc.sync.dma_start(y[:, ht, c0 : c0 + cs], yv[:, ht, :cs])

            # software pipeline: stage 1 runs a chunk ahead so stage-2 has
            # its H planes ready and the w2 stream time to land
            stage1(0)
            for ci in range(1, n_chunks):
                stage1(ci)
                stage2(ci - 1)
            stage2(n_chunks - 1)
    nc.compile()
    global LAST_NC
    LAST_NC = nc
    return nc


def route(router_logits):
    """Host-side router: softmax -> top-2 -> renormalize."""
    logits = np.asarray(router_logits, dtype=np.float32)
    m = logits.max(axis=-1, keepdims=True)
    ex = np.exp(logits - m)
    probs = ex / ex.sum(axis=-1, keepdims=True)
    order = np.argsort(-probs, axis=-1, kind="stable")[:, :TOPK]
    rows = np.arange(logits.shape[0])[:, None]
    topk_p = probs[rows, order]
    topk_p = topk_p / topk_p.sum(axis=-1, keepdims=True)
    return order, topk_p.astype(np.float32)


def _split_e4(a):
    """Residual split: a ~= hi + lo, both e4m3 (RN residual is exact)."""
    hi = np.asarray(a, np.float32).astype(E4NP)
    lo = (a - hi.astype(np.float32)).astype(E4NP)
    return hi, lo


def kernel(x, router_logits, w1, w2):
    x = np.ascontiguousarray(np.asarray(x, dtype=np.float32))
    w1 = np.asarray(w1, dtype=np.float32)
    w2 = np.asarray(w2, dtype=np.float32)
    t = x.shape[0]

    top2_idx, top2_gate = route(router_logits)

    expert_tokens = []
    expert_gates = []
    for e in range(E):
        sel = np.nonzero(top2_idx == e)
        expert_tokens.append(sel[0])
        expert_gates.append(top2_gate[sel[0], sel[1]])
    counts = [len(ix) for ix in expert_tokens]
    count = max(2, max(counts) + max(counts) % 2)
    C = count

    nc = build_moe_expert_kernel(count)

    in_maps = []
    for e in range(E):
        cnt = counts[e]
        xT = np.zeros((H, C), dtype=np.float32)
        xT[:, :cnt] = x[expert_tokens[e]].T
        Xh, Xl = _split_e4(SX * xT)
        # two planes [H, C] -> [P, 2, HK, C]
        x_a = np.ascontiguousarray(
            np.stack([Xh, Xl]).reshape(2, HK, P, C).transpose(2, 0, 1, 3)
        )

        w1T = w1[e].T  # [H, I]
        W1h, W1l = _split_e4(SW1 * w1T)
        # two planes [H, I] -> [IT, P, 2, HK, P]
        w1_a = np.ascontiguousarray(
            np.stack([W1h, W1l]).reshape(2, HK, P, IT, P).transpose(3, 2, 0, 1, 4)
        )

        w2T = np.zeros((IT2 * P, H), dtype=np.float32)  # [I padded, H]
        w2T[:I] = w2[e].T
        W2h, W2l = _split_e4(SW2 * w2T)
        # two planes [Ipad, H] -> [HK, P, 2, IT2, P]
        w2_a = np.ascontiguousarray(
            np.stack([W2h, W2l]).reshape(2, IT2, P, HK, P).transpose(3, 2, 0, 1, 4)
        )

        in_maps.append({"x": x_a, "w1": w1_a, "w2": w2_a})

    res = run_bass_kernel_spmd(nc, in_maps, core_ids=list(range(N_CORES)))
    ys = [np.asarray(r["y"], dtype=np.float32) for r in res.results]
    if not all(np.isfinite(yy).all() for yy in ys):
        # one retry in case of a transient device fault
        res = run_bass_kernel_spmd(nc, in_maps, core_ids=list(range(N_CORES)))
        ys = [np.asarray(r["y"], dtype=np.float32) for r in res.results]

    out = np.zeros((t, H), dtype=np.float32)
    for e in range(E):
        cnt = counts[e]
        # y dram [P, HK, C] -> [C, H]
        y_e = ys[e].transpose(2, 1, 0).reshape(C, H)
        g = expert_gates[e][:, None] * (1.0 / PS2_SCALE)
        out[expert_tokens[e]] += g * y_e[:cnt]
    return out


# revision 28
# speedup vs baseline: 1.0185x; 1.0185x over previous
"""MoE (top-2 of 8 experts) Trainium2 kernel — fp8 DoubleRow edition.

Strategy: expert-parallel across the 8 NeuronCores (host router builds the
dispatch; core e gets the tokens routed to expert e). The expert MLP runs
as 3-term residual-split fp8 matmuls in DoubleRow perf mode (contraction
256 per pass, 0.5 PE cycles per output column — 4x the bf16 MAC rate):

  A ≈ Ah + Al  (Ah = e4m3(s*A), Al = e4m3(s*A - Ah): the RN residual is
  exactly representable in e4m3, so Ah+Al carries ~9 significant bits)

  A@B ≈ Ah@Bh + Ah@Bl + Al@Bh   (lo*lo term dropped, ~1e-3 rel)

Per-core pipeline (count tokens, chunks of <=512 on the free axis):
  stage 1: ps1 = 16*z = sum of 3 DoubleRow terms (w1 planes x x planes)
           sg = sigmoid(ps1/16)         [ACT]
           t  = ps1*sg = 16*silu(z)     [DVE, bf16]
           Hh = e4m3(t)                 [ACT copy]
           Hl = e4m3(t - Hh)            [DVE fused (t*1)-Hh]
  stage 2: ps2 = 512*y = 3 DoubleRow terms (w2 planes x H planes),
           I padded to 1536 (12 i-tiles) with zero weights/h
           yv = bf16(ps2)               [ACT copy] -> DMA out
  host:    out[tok] += gate/512 * yv.T  (gate multiply + dequant on host)

All DRAM operands are host-tiled so every DMA lands with >=512B
contiguous runs and block granularity matching PE consumption order.
"""

import numpy as np
import ml_dtypes

import concourse.mybir as mybir
from concourse import bacc
from concourse.tile import TileContext
from concourse.bass_utils import run_bass_kernel_spmd

T, H, I, E = 4096, 1024, 1408, 8
TOPK = 2
P = 128
HK = H // P  # 8 h-tiles (stage-1 contraction, stage-2 output)
IT = I // P  # 11 i-tiles (stage-1 output)
IT2 = IT + 1  # stage-2 contraction padded to 12 tiles (6 DoubleRow pairs)
CHUNK = 512
N_CORES = 8

F32 = mybir.dt.float32
BF16 = mybir.dt.bfloat16
F8 = mybir.dt.float8e4
AF = mybir.ActivationFunctionType
DR = mybir.MatmulPerfMode.DoubleRow
E4NP = ml_dtypes.float8_e4m3

WARMUP_MM = 16  # PE warm-up matmuls (see build_moe_expert_kernel)
SX = 2.0  # x plane scale
SW1 = 8.0  # w1 plane scale ->  ps1 = 16*z
SW2 = 32.0  # w2 plane scale ->  ps2 = 512*y
PS1_SCALE = SX * SW1
PS2_SCALE = PS1_SCALE * SW2

# most recently built device program (for test harnesses / cost-model timing)
LAST_NC = None


def _chunks(count):
    out = []
    r = count
    while r > 0:
        c = min(CHUNK, r)
        out.append(c)
        r -= c
    # keep a full-size chunk LAST: its stage-2 compute covers the per-ht
    # output-DMA issue slots, so only one small DMA sits in the tail
    if len(out) >= 2 and out[-1] < CHUNK:
        out[-1], out[-2] = out[-2], out[-1]
    return out


def build_moe_expert_kernel(count):
    """One-expert MLP over `count` gathered tokens (count even)."""
    C = count
    assert count % 2 == 0
    c_chunks = _chunks(count)
    c_starts = [sum(c_chunks[:j]) for j in range(len(c_chunks))]
    n_chunks = len(c_chunks)

    nc = bacc.Bacc("TRN2", target_bir_lowering=False, debug=False, num_devices=N_CORES)
    # hi/lo fp8 planes packed in one tensor each: a single DMA delivers
    # both (HWDGE issue slots, 625ns apiece, are the scarce resource)
    x = nc.dram_tensor("x", [P, 2, HK, C], F8, kind="ExternalInput").ap()
    w1 = nc.dram_tensor("w1", [IT, P, 2, HK, P], F8, kind="ExternalInput").ap()
    w2 = nc.dram_tensor("w2", [HK, P, 2, IT2, P], F8, kind="ExternalInput").ap()
    y = nc.dram_tensor("y", [P, HK, C], BF16, kind="ExternalOutput").ap()

    with TileContext(nc) as tc:
        with (
            tc.tile_pool(name="wpool", bufs=1) as wpool,
            tc.tile_pool(name="xpool", bufs=min(3, n_chunks)) as xpool,
            tc.tile_pool(name="hpool", bufs=n_chunks) as hpool,
            tc.tile_pool(name="spool", bufs=4) as spool,
            tc.tile_pool(name="ypool", bufs=min(2, n_chunks)) as ypool,
            tc.tile_pool(name="ps1", bufs=4, space="PSUM") as ps1pool,
            tc.tile_pool(name="ps2", bufs=4, space="PSUM") as ps2pool,
        ):
            w1s = wpool.tile([P, IT, 2, HK, P], F8)
            w2s = wpool.tile([P, HK, 2, IT2, P], F8)
            # scratch operand for PE warm-up matmuls (zeroed once by DVE)
            wu = wpool.tile([P, 2, CHUNK], F8)

            x_t = {}

            def load_x(ci, slabs=False):
                # per-plane DMAs: a plane slice keeps the access pattern
                # within the 3-dim DMA balancing limit
                cs, c0 = c_chunks[ci], c_starts[ci]
                tx = xpool.tile([P, 2, HK, CHUNK], F8, tag="xs", name=f"xs{ci}")
                if slabs:
                    # hk-halves per plane, interleaved, so the first
                    # stage-1 group's terms unblock progressively
                    for k in range(2):
                        sl = slice(4 * k, 4 * k + 4)
                        nc.sync.dma_start(
                            tx[:, 0, sl, :cs], x[:, 0, sl, c0 : c0 + cs]
                        )
                        nc.sync.dma_start(
                            tx[:, 1, sl, :cs], x[:, 1, sl, c0 : c0 + cs]
                        )
                else:
                    for pl in range(2):
                        nc.sync.dma_start(
                            tx[:, pl, :, :cs], x[:, pl, :, c0 : c0 + cs]
                        )
                x_t[ci] = tx

            # DMA issue order = consumption order: w1 it0, the first x
            # chunk in slabs, the w1 stream (later x chunks slotted in
            # where the stall-free window allows), then w2.
            nc.sync.dma_start(w1s[:, 0, :, :, :], w1[0])
            load_x(0, slabs=True)
            for it in range(1, IT):
                nc.sync.dma_start(w1s[:, it, :, :, :], w1[it])
                if it == 4:
                    for ci in range(1, n_chunks):
                        load_x(ci)
            for ht in range(HK):
                nc.sync.dma_start(w2s[:, ht, :, :, :], w2[ht])

            # PE p-state warm-up: the cost model runs the PE at reduced
            # clock for the first ~3us of a busy period. Dummy matmuls on
            # scratch data (discarded psum) keep the PE continuously busy
            # from t~0 so the ramp completes before real operands land.
            nc.vector.memset(wu[:, :, :], 0.0)
            wu_ps = ps1pool.tile([P, CHUNK], F32, tag="ps1")
            for _ in range(WARMUP_MM):
                nc.tensor.matmul(
                    wu_ps[:, :CHUNK],
                    wu[:, :, :P],
                    wu[:, :, :],
                    start=True,
                    stop=True,
                    perf_mode=DR,
                )

            hh_t, hl_t = {}, {}

            def stage1(ci):
                cs = c_chunks[ci]
                xs = x_t[ci]
                hh = hpool.tile([P, IT2, CHUNK], F8, tag="hh", name=f"hh{ci}")
                hl = hpool.tile([P, IT2, CHUNK], F8, tag="hl", name=f"hl{ci}")
                # stage-2 reads the zero-pad i-tile via its DoubleRow pair
                nc.gpsimd.memset(hh[:, IT, :cs], 0.0)
                nc.gpsimd.memset(hl[:, IT, :cs], 0.0)
                for it in range(IT):
                    ps1 = ps1pool.tile([P, CHUNK], F32, tag="ps1")
                    terms = ((0, 0), (0, 1), (1, 0))  # (w-plane, x-plane)
                    for ti, (wp, xp) in enumerate(terms):
                        for k in range(HK // 2):
                            nc.tensor.matmul(
                                ps1[:, :cs],
                                w1s[:, it, wp, 2 * k : 2 * k + 2, :],
                                xs[:, xp, 2 * k : 2 * k + 2, :cs],
                                start=(ti == 0 and k == 0),
                                stop=(ti == 2 and k == HK // 2 - 1),
                                perf_mode=DR,
                            )
                    sg = spool.tile([P, CHUNK], F32, tag="sg")
                    t = spool.tile([P, CHUNK], BF16, tag="t")
                    nc.scalar.activation(
                        sg[:, :cs], ps1[:, :cs], AF.Sigmoid, scale=1.0 / PS1_SCALE
                    )
                    nc.vector.tensor_mul(out=t[:, :cs], in0=ps1[:, :cs], in1=sg[:, :cs])
                    nc.scalar.activation(hh[:, it, :cs], t[:, :cs], AF.Copy)
                    # residual on GPSIMD: keeps the DVE (which paces ps1
                    # reuse) well under the PE rate during stage 1
                    nc.gpsimd.scalar_tensor_tensor(
                        out=hl[:, it, :cs],
                        in0=t[:, :cs],
                        scalar=1.0,
                        in1=hh[:, it, :cs],
                        op0=mybir.AluOpType.mult,
                        op1=mybir.AluOpType.subtract,
                    )
                hh_t[ci], hl_t[ci] = hh, hl

            def stage2(ci):
                cs, c0 = c_chunks[ci], c_starts[ci]
                hh, hl = hh_t.pop(ci), hl_t.pop(ci)
                yv = ypool.tile([P, HK, CHUNK], BF16, tag="yv", name=f"yv{ci}")
                for ht in range(HK):
                    ps2 = ps2pool.tile([P, CHUNK], F32, tag="ps2")
                    terms = ((0, hh), (0, hl), (1, hh))  # (w-plane, h-plane)
                    for ti, (wp, hs) in enumerate(terms):
                        for k in range(IT2 // 2):
                            nc.tensor.matmul(
                                ps2[:, :cs],
                                w2s[:, ht, wp, 2 * k : 2 * k + 2, :],
                                hs[:, 2 * k : 2 * k + 2, :cs],
                                start=(ti == 0 and k == 0),
                                stop=(ti == 2 and k == IT2 // 2 - 1),
                                perf_mode=DR,
                            )
                    nc.scalar.activation(yv[:, ht, :cs], ps2[:, :cs], AF.Copy)
                    # per-ht output DMAs drain under the remaining groups
                    nc.sync.dma_start(y[:, ht, c0 : c0 + cs], yv[:, ht, :cs])

            # all stage-1 passes, then all stage-2 passes: every chunk's
            # silu/split finalize chain gets a full pass of PE cover, and
            # the w2 stream has the whole stage-1 phase to land
            for ci in range(n_chunks):
                stage1(ci)
            for ci in range(n_chunks):
                stage2(ci)
    nc.compile()
    global LAST_NC
    LAST_NC = nc
    return nc


def route(router_logits):
    """Host-side router: softmax -> top-2 -> renormalize."""
    logits = np.asarray(router_logits, dtype=np.float32)
    m = logits.max(axis=-1, keepdims=True)
    ex = np.exp(logits - m)
    probs = ex / ex.sum(axis=-1, keepdims=True)
    order = np.argsort(-probs, axis=-1, kind="stable")[:, :TOPK]
    rows = np.arange(logits.shape[0])[:, None]
    topk_p = probs[rows, order]
    topk_p = topk_p / topk_p.sum(axis=-1, keepdims=True)
    return order, topk_p.astype(np.float32)


def _split_e4(a):
    """Residual split: a ~= hi + lo, both e4m3 (RN residual is exact)."""
    hi = np.asarray(a, np.float32).astype(E4NP)
    lo = (a - hi.astype(np.float32)).astype(E4NP)
    return hi, lo


def kernel(x, router_logits, w1, w2):
    x = np.ascontiguousarray(np.asarray(x, dtype=np.float32))
    w1 = np.asarray(w1, dtype=np.float32)
    w2 = np.asarray(w2, dtype=np.float32)
    t = x.shape[0]

    top2_idx, top2_gate = route(router_logits)

    expert_tokens = []
    expert_gates = []
    for e in range(E):
        sel = np.nonzero(top2_idx == e)
        expert_tokens.append(sel[0])
        expert_gates.append(top2_gate[sel[0], sel[1]])
    counts = [len(ix) for ix in expert_tokens]
    count = max(2, max(counts) + max(counts) % 2)
    C = count

    nc = build_moe_expert_kernel(count)

    in_maps = []
    for e in range(E):
        cnt = counts[e]
        xT = np.zeros((H, C), dtype=np.float32)
        xT[:, :cnt] = x[expert_tokens[e]].T
        Xh, Xl = _split_e4(SX * xT)
        # two planes [H, C] -> [P, 2, HK, C]
        x_a = np.ascontiguousarray(
            np.stack([Xh, Xl]).reshape(2, HK, P, C).transpose(2, 0, 1, 3)
        )

        w1T = w1[e].T  # [H, I]
        W1h, W1l = _split_e4(SW1 * w1T)
        # two planes [H, I] -> [IT, P, 2, HK, P]
        w1_a = np.ascontiguousarray(
            np.stack([W1h, W1l]).reshape(2, HK, P, IT, P).transpose(3, 2, 0, 1, 4)
        )

        w2T = np.zeros((IT2 * P, H), dtype=np.float32)  # [I padded, H]
        w2T[:I] = w2[e].T
        W2h, W2l = _split_e4(SW2 * w2T)
        # two planes [Ipad, H] -> [HK, P, 2, IT2, P]
        w2_a = np.ascontiguousarray(
            np.stack([W2h, W2l]).reshape(2, IT2, P, HK, P).transpose(3, 2, 0, 1, 4)
        )

        in_maps.append({"x": x_a, "w1": w1_a, "w2": w2_a})

    res = run_bass_kernel_spmd(nc, in_maps, core_ids=list(range(N_CORES)))
    ys = [np.asarray(r["y"], dtype=np.float32) for r in res.results]
    if not all(np.isfinite(yy).all() for yy in ys):
        # one retry in case of a transient device fault
        res = run_bass_kernel_spmd(nc, in_maps, core_ids=list(range(N_CORES)))
        ys = [np.asarray(r["y"], dtype=np.float32) for r in res.results]

    out = np.zeros((t, H), dtype=np.float32)
    for e in range(E):
        cnt = counts[e]
        # y dram [P, HK, C] -> [C, H]
        y_e = ys[e].transpose(2, 1, 0).reshape(C, H)
        g = expert_gates[e][:, None] * (1.0 / PS2_SCALE)
        out[expert_tokens[e]] += g * y_e[:cnt]
    return out
